# revision 34
# baseline (speedup 1.0000x reference)
"""Trainium2 Bass kernel: pre-LN multi-head attention block (B=8, L=1024,
D=1024, H=16, dk=dv=64), data-parallel over batch across 8 NeuronCores.

Per core (one batch element):
  qn   = LN(q) ; QT = (Wq/8)^T-proj feature-major ; KT likewise ; V token-major
  S^T  = K_h Q_h^T per head (feature-major, softmax dim on partitions,
         head pairs row-packed onto PE row groups 0-63 / 64-127)
  P^T  = exp(S^T)               (no max-subtraction needed: |S| <~ 7)
  O^T  = V_aug^T P^T            (ones column in V_aug -> sumexp row for free)
  O    = O^T / sumexp           (approx-recip bcast via 1-row PE matmul)
  out  = LN(O @ Wo + q)
"""

import numpy as np
import ml_dtypes

import concourse.bass as bass
import concourse.mybir as mybir
import concourse.tile as tile
from concourse import bacc
from concourse.dve_ops import RECIP_APPROX_FAST_CONSTS, RECIPROCAL_APPROX_FAST

P = 128
L = 1024          # tokens per batch element
D = 1024          # model dim
H = 16            # heads
HD = 64           # head dim
E = HD + 1        # head dim + sumexp column
NC = D // P       # 8 feature chunks
NT = L // P       # 8 token chunks
NQ = 2            # halves of the q/free dimension
QH = 512
EPS = 1e-6

FP32 = mybir.dt.float32
BF16 = mybir.dt.bfloat16
FP32R = mybir.dt.float32r
AX = mybir.AxisListType.X
OP = mybir.AluOpType
AF = mybir.ActivationFunctionType


def _emit_layernorm(nc, pool, x, y, gamma_bc, beta_bc, eps_t):
    """y = LN(x) * gamma + beta for one [P, D] token-major tile.

    Row sums via ACT accum_out (y doubles as the ACT scratch output);
    per-partition stats and the gamma/beta elementwise tail on DVE.
    """
    st = pool.tile([P, 8], FP32, tag="lnst", bufs=3, name="lnst")
    nc.scalar.activation(y, x, AF.Copy, accum_out=st[:, 0:1])       # sum x
    nc.scalar.activation(y, x, AF.Square, accum_out=st[:, 1:2])     # sum x^2
    nc.vector.tensor_scalar_mul(st[:, 2:3], st[:, 0:1], 1.0 / D)    # mu
    nc.vector.tensor_tensor(st[:, 3:4], st[:, 2:3], st[:, 2:3], OP.mult)
    nc.vector.tensor_scalar_mul(st[:, 4:5], st[:, 1:2], 1.0 / D)    # E[x^2]
    nc.vector.tensor_tensor(st[:, 4:5], st[:, 4:5], st[:, 3:4], OP.subtract)
    nc.scalar.activation(st[:, 5:6], st[:, 4:5], AF.Sqrt, bias=eps_t)
    nc.vector.reciprocal(st[:, 6:7], st[:, 5:6])                    # rstd
    nc.vector.tensor_tensor(st[:, 7:8], st[:, 2:3], st[:, 6:7], OP.mult)
    nc.vector.tensor_scalar_mul(st[:, 7:8], st[:, 7:8], -1.0)       # -mu*rstd
    # xn = x*rstd - mu*rstd on ACT (per-partition scale/bias APs)
    nc.scalar.activation(y, x, AF.Identity, bias=st[:, 7:8], scale=st[:, 6:7])
    nc.vector.tensor_tensor(y, y, gamma_bc, OP.mult)
    nc.vector.tensor_tensor(y, y, beta_bc, OP.add)


def build_bass():
    nc = bacc.Bacc("TRN2", target_bir_lowering=False, debug=False)

    q_d = nc.dram_tensor("q", [L, D], FP32, kind="ExternalInput")
    k_d = nc.dram_tensor("k", [L, D], FP32, kind="ExternalInput")
    v_d = nc.dram_tensor("v", [L, D], FP32, kind="ExternalInput")
    wq_d = nc.dram_tensor("wq", [D, D], FP32R, kind="ExternalInput")
    wk_d = nc.dram_tensor("wk", [D, D], FP32R, kind="ExternalInput")
    wv_d = nc.dram_tensor("wv", [D, D], FP32R, kind="ExternalInput")
    wo_d = nc.dram_tensor("wo", [D, D], BF16, kind="ExternalInput")
    gb_d = nc.dram_tensor("gb", [P, D], FP32, kind="ExternalInput")
    bb_d = nc.dram_tensor("bb", [P, D], FP32, kind="ExternalInput")
    id_d = nc.dram_tensor("ident", [P, P], FP32, kind="ExternalInput")
    on_d = nc.dram_tensor("ones1", [1, HD], FP32R, kind="ExternalInput")
    ep_d = nc.dram_tensor("epsc", [P, 1], FP32, kind="ExternalInput")
    vo_d = nc.dram_tensor("vone", [P, H * E], BF16, kind="ExternalInput")
    zz_d = nc.dram_tensor("zz", [HD, L], FP32R, kind="ExternalInput")
    out_d = nc.dram_tensor("out", [L, D], FP32, kind="ExternalOutput")

    with tile.TileContext(nc) as tc:
        with (
            tc.tile_pool(name="const", bufs=1) as cpool,
            tc.tile_pool(name="otp", bufs=1) as otp,
        ):
            gamma_bc = cpool.tile([P, D], FP32, name="gamma_bc")
            beta_bc = cpool.tile([P, D], FP32, name="beta_bc")
            ident = cpool.tile([P, P], FP32, name="ident")
            eps_t = cpool.tile([P, 1], FP32, name="eps_t")
            ones1 = cpool.tile([1, HD], FP32R, name="ones1")
            nc.gpsimd.dma_start(gamma_bc, gb_d[:])
            nc.gpsimd.dma_start(beta_bc, bb_d[:])
            nc.gpsimd.dma_start(ident, id_d[:])
            nc.gpsimd.dma_start(eps_t, ep_d[:])
            nc.gpsimd.dma_start(ones1, on_d[:])

            def load_w(pool, dram, dtype):
                tiles = []
                for i in range(NC):
                    wt = pool.tile([P, D], dtype, tag="w", bufs=8,
                                   name=f"w{i}")
                    nc.sync.dma_start(wt, dram[i * P:(i + 1) * P, :])
                    tiles.append(wt)
                return tiles

            OT = [otp.tile([P, L], BF16, tag="ot", bufs=8, name=f"ot{j}")
                  for j in range(H // 2)]

            with tc.tile_pool(name="qkv", bufs=1) as qkv:
                QT = qkv.tile([P, NC, L], FP32R, tag="QT", name="QT")
                KT = qkv.tile([P, NC, L], FP32R, tag="KT", name="KT")
                Vaug = qkv.tile([P, NT, H * E], BF16, tag="Vaug", name="Vaug")

                with (
                    tc.tile_pool(name="actT", bufs=1) as atp,
                    tc.tile_pool(name="ps1", bufs=6, space="PSUM") as ps1,
                ):

                    def load_transpose(src_d, do_ln):
                        """Load token-major [L, D], optionally LN, and build
                        8 feature-major [P, L] chunks."""
                        chunks = [atp.tile([P, L], FP32R, tag="actT", bufs=8,
                                           name=f"tchunk{c}")
                                  for c in range(NC)]
                        for t in range(NT):
                            x = atp.tile([P, D], FP32, tag="in", bufs=2,
                                         name="x_in")
                            nc.sync.dma_start(x, src_d[t * P:(t + 1) * P, :])
                            if do_ln:
                                y = atp.tile([P, D], FP32, tag="qn", bufs=6,
                                             name="x_ln")
                                _emit_layernorm(nc, atp, x, y, gamma_bc,
                                                beta_bc, eps_t)
                                x = y
                            for c in range(NC):
                                pt = ps1.tile([P, P], FP32, tag="mm",
                                              name="ps_tr")
                                nc.tensor.transpose(
                                    pt, x[:, c * P:(c + 1) * P], ident)
                                nc.vector.tensor_copy(
                                    chunks[c][:, t * P:(t + 1) * P], pt)
                        return chunks

                    def proj_featmajor(w_tiles, act_chunks, OUTT):
                        """OUTT[:, m, :] = (W^T @ actT) feature-major."""
                        for m in range(NC):
                            for n in range(NQ):
                                ps = ps1.tile([P, QH], FP32, tag="mm",
                                              name="ps_pj")
                                for i in range(NC):
                                    nc.tensor.matmul(
                                        ps,
                                        w_tiles[i][:, m * P:(m + 1) * P],
                                        act_chunks[i][:, n * QH:(n + 1) * QH],
                                        start=(i == 0), stop=(i == NC - 1),
                                    )
                                nc.vector.tensor_copy(
                                    OUTT[:, m, n * QH:(n + 1) * QH], ps)

                    # --- k -> kT -> KT ---
                    kT = load_transpose(k_d, do_ln=False)
                    wk_t = load_w(atp, wk_d, FP32R)
                    proj_featmajor(wk_t, kT, KT)

                    # --- q -> LN -> qnT -> QT ---
                    qnT = load_transpose(q_d, do_ln=True)
                    wq_t = load_w(atp, wq_d, FP32R)
                    proj_featmajor(wq_t, qnT, QT)

                    # --- v -> vT -> V (token-major) -> Vaug ---
                    for t in range(NT):
                        nc.sync.dma_start(Vaug[:, t, :], vo_d[:])
                    vT = load_transpose(v_d, do_ln=False)
                    wv_t = load_w(atp, wv_d, FP32R)
                    for t in range(NT):
                        for n in range(NQ):
                            ps = ps1.tile([P, QH], FP32, tag="mm",
                                          name="ps_v")
                            for i in range(NC):
                                nc.tensor.matmul(
                                    ps,
                                    vT[i][:, t * P:(t + 1) * P],
                                    wv_t[i][:, n * QH:(n + 1) * QH],
                                    start=(i == 0), stop=(i == NC - 1),
                                )
                            dst = Vaug[:, t, n * 8 * E:(n + 1) * 8 * E]
                            dst = dst.rearrange("p (h e) -> p h e", e=E)
                            nc.vector.tensor_copy(
                                dst[:, :, 0:HD],
                                ps.rearrange("p (h d) -> p h d", d=HD))

                # ---------------- attention ----------------
                with (
                    tc.tile_pool(name="att", bufs=1) as att,
                    tc.tile_pool(name="ps2", bufs=1, space="PSUM") as ps2,
                ):
                    # zero-padded K^T copies: full-128 contraction keeps the
                    # PE array fully active (S matmuls shaped like the
                    # projection matmuls -> pipelined + HAM-warm)
                    KTza = att.tile([P, NC, L], FP32R, tag="ktz", bufs=2,
                                    name="KTza")
                    KTzb = att.tile([P, NC, L], FP32R, tag="ktz", bufs=2,
                                    name="KTzb")
                    for c in range(NC):
                        nc.sync.dma_start(KTza[HD:P, c, :], zz_d[:])
                        nc.sync.dma_start(KTzb[0:HD, c, :], zz_d[:])
                        nc.vector.tensor_copy(KTza[0:HD, c, :],
                                              KT[0:HD, c, :])
                        nc.vector.tensor_copy(KTzb[HD:P, c, :],
                                              KT[HD:P, c, :])
                    for j in range(H // 2):
                        ha, hb = 2 * j, 2 * j + 1
                        for n in range(NQ):
                            qs = slice(n * QH, (n + 1) * QH)
                            PTa = att.tile([P, NC, QH], BF16, tag="pt",
                                           bufs=3, name="pta")
                            PTb = att.tile([P, NC, QH], BF16, tag="pt",
                                           bufs=3, name="ptb")
                            for i in range(NC):
                                ks = slice(i * P, (i + 1) * P)
                                psa = ps2.tile([P, QH], FP32, tag="s",
                                               bufs=5, name="psa")
                                psb = ps2.tile([P, QH], FP32, tag="s",
                                               bufs=5, name="psb")
                                nc.tensor.matmul(
                                    psa, KTza[:, j, ks], QT[:, j, qs],
                                    start=True, stop=True)
                                nc.tensor.matmul(
                                    psb, KTzb[:, j, ks], QT[:, j, qs],
                                    start=True, stop=True)
                                nc.scalar.activation(PTa[:, i, :], psa, AF.Exp)
                                nc.scalar.activation(PTb[:, i, :], psb, AF.Exp)
                            poa = ps2.tile([E, QH], FP32, tag="o", bufs=2,
                                           name="poa")
                            pob = ps2.tile([E, QH], FP32, tag="o", bufs=2,
                                           name="pob")
                            for i in range(NC):
                                nc.tensor.matmul(
                                    poa, Vaug[:, i, ha * E:(ha + 1) * E],
                                    PTa[:, i, :],
                                    start=(i == 0), stop=(i == NC - 1))
                                nc.tensor.matmul(
                                    pob, Vaug[:, i, hb * E:(hb + 1) * E],
                                    PTb[:, i, :],
                                    start=(i == 0), stop=(i == NC - 1))
                            for sub, po in ((0, poa), (1, pob)):
                                o_tmp = att.tile([E, QH], FP32, tag="otmp",
                                                 bufs=2, name="o_tmp")
                                nc.vector.tensor_copy(o_tmp, po)
                                rin = att.tile([1, QH], FP32, tag="rin",
                                               bufs=2, name="rin")
                                nc.sync.dma_start(rin, o_tmp[HD:E, :])
                                rec = att.tile([1, QH], FP32R, tag="rec",
                                               bufs=2, name="rec")
                                c = RECIP_APPROX_FAST_CONSTS
                                nc.vector._custom_dve(
                                    RECIPROCAL_APPROX_FAST, out=rec, in0=rin,
                                    s0=c["s0"], s1=c["s1"], imm2=c["imm2"])
                                pbc = ps2.tile([HD, QH], FP32, tag="bc",
                                               bufs=1, name="pbc")
                                nc.tensor.matmul(pbc, ones1, rec,
                                                 start=True, stop=True)
                                if sub == 0:
                                    nc.vector.tensor_tensor(
                                        OT[j][0:HD, qs], o_tmp[0:HD, :],
                                        pbc, OP.mult)
                                else:
                                    oo = att.tile([HD, QH], BF16, tag="oo",
                                                  bufs=2, name="oo")
                                    nc.vector.tensor_tensor(
                                        oo, o_tmp[0:HD, :], pbc, OP.mult)
                                    nc.sync.dma_start(OT[j][HD:P, qs], oo)

            # ---------------- output projection + transpose + LN ---------
            with (
                tc.tile_pool(name="fin", bufs=1) as fin,
                tc.tile_pool(name="ps3", bufs=1, space="PSUM") as ps3,
            ):
                wo_t = load_w(fin, wo_d, BF16)
                res = []
                for t in range(NT):
                    rt = fin.tile([P, D], FP32, tag="res", bufs=8,
                                  name=f"res{t}")
                    nc.sync.dma_start(rt, q_d[t * P:(t + 1) * P, :])
                    res.append(rt)
                out_acc = [fin.tile([P, D], FP32, tag="oacc", bufs=8,
                                    name=f"oacc{t}")
                           for t in range(NT)]

                def emit_tr(m, pj):
                    """Transpose projT_m blocks into out_acc, fusing the
                    residual add."""
                    for t in range(NT):
                        pt = ps3.tile([P, P], FP32, tag="tr", bufs=4,
                                      name="ps_tr2")
                        nc.tensor.transpose(
                            pt, pj[:, t * P:(t + 1) * P], ident)
                        ms = slice(m * P, (m + 1) * P)
                        nc.vector.tensor_tensor(
                            out_acc[t][:, ms], pt, res[t][:, ms], OP.add)

                prev = None
                for m in range(NC):
                    pj = fin.tile([P, L], FP32, tag="pjt", bufs=3,
                                  name="pjt")
                    for n in range(NQ):
                        ps = ps3.tile([P, QH], FP32, tag="mm", bufs=4,
                                      name="ps_w")
                        for jj in range(NC):
                            nc.tensor.matmul(
                                ps,
                                wo_t[jj][:, m * P:(m + 1) * P],
                                OT[jj][:, n * QH:(n + 1) * QH],
                                start=(jj == 0), stop=(jj == NC - 1),
                            )
                        nc.vector.tensor_copy(pj[:, n * QH:(n + 1) * QH], ps)
                    if prev is not None:
                        emit_tr(m - 1, prev)
                    prev = pj
                emit_tr(NC - 1, prev)

                for t in range(NT):
                    u = out_acc[t]
                    y = fin.tile([P, D], FP32, tag="y", bufs=2, name="y")
                    _emit_layernorm(nc, fin, u, y, gamma_bc, beta_bc, eps_t)
                    nc.sync.dma_start(out_d[t * P:(t + 1) * P, :], y)

    nc.compile()
    return nc


_CACHE = {}


def _get_nc():
    if "nc" not in _CACHE:
        _CACHE["nc"] = build_bass()
    return _CACHE["nc"]


def make_in_maps(q, k, v, Wq, Wk, Wv, Wo, gamma, beta):
    q = np.asarray(q, np.float32)
    k = np.asarray(k, np.float32)
    v = np.asarray(v, np.float32)
    # fold the 1/sqrt(dk) attention scale into Wq (0.125 is exact in fp32)
    wq = (np.asarray(Wq, np.float32) * 0.125).astype(np.float32)
    wk = np.ascontiguousarray(np.asarray(Wk, np.float32))
    wv = np.ascontiguousarray(np.asarray(Wv, np.float32))
    wo = np.asarray(Wo, np.float32).astype(ml_dtypes.bfloat16)
    gb = np.ascontiguousarray(
        np.tile(np.asarray(gamma, np.float32)[None, :], (P, 1)))
    bb = np.ascontiguousarray(
        np.tile(np.asarray(beta, np.float32)[None, :], (P, 1)))
    ident = np.eye(P, dtype=np.float32)
    ones1 = np.ones((1, HD), np.float32)
    epsc = np.full((P, 1), EPS, np.float32)
    vone = np.ones((P, H * E), ml_dtypes.bfloat16)
    zz = np.zeros((HD, L), np.float32)
    B = q.shape[0]
    return [
        {
            "q": np.ascontiguousarray(q[b]),
            "k": np.ascontiguousarray(k[b]),
            "v": np.ascontiguousarray(v[b]),
            "wq": wq, "wk": wk, "wv": wv, "wo": wo,
            "gb": gb, "bb": bb, "ident": ident, "ones1": ones1,
            "epsc": epsc, "vone": vone, "zz": zz,
        }
        for b in range(B)
    ]


def kernel(q, k, v, Wq, Wk, Wv, Wo, gamma, beta, trace=False):
    from concourse.bass_utils import run_bass_kernel_spmd

    nc = _get_nc()
    in_maps = make_in_maps(q, k, v, Wq, Wk, Wv, Wo, gamma, beta)
    res = run_bass_kernel_spmd(nc, in_maps, core_ids=list(range(len(in_maps))),
                               trace=trace)
    out = np.stack([r["out"] for r in res.results], axis=0)
    if trace:
        return out, res
    return out


# revision 35
# speedup vs baseline: 1.1685x; 1.1685x over previous
"""Trainium2 Bass kernel: pre-LN multi-head attention block (B=8, L=1024,
D=1024, H=16, dk=dv=64), data-parallel over batch across 8 NeuronCores.

Per core (one batch element):
  qn   = LN(q) ; QT = (Wq/8)^T-proj feature-major ; KT likewise ; V token-major
  S^T  = K_h Q_h^T per head (feature-major, softmax dim on partitions,
         head pairs row-packed onto PE row groups 0-63 / 64-127)
  P^T  = exp(S^T)               (no max-subtraction needed: |S| <~ 7)
  O^T  = V_aug^T P^T            (ones column in V_aug -> sumexp row for free)
  O    = O^T / sumexp           (approx-recip bcast via 1-row PE matmul)
  out  = LN(O @ Wo + q)
"""

import numpy as np
import ml_dtypes

import concourse.bass as bass
import concourse.mybir as mybir
import concourse.tile as tile
from concourse import bacc
from concourse.dve_ops import RECIP_APPROX_FAST_CONSTS, RECIPROCAL_APPROX_FAST

P = 128
L = 1024          # tokens per batch element
D = 1024          # model dim
H = 16            # heads
HD = 64           # head dim
E = HD + 1        # head dim + sumexp column
NC = D // P       # 8 feature chunks
NT = L // P       # 8 token chunks
NQ = 2            # halves of the q/free dimension
QH = 512
EPS = 1e-6

FP32 = mybir.dt.float32
BF16 = mybir.dt.bfloat16
FP32R = mybir.dt.float32r
AX = mybir.AxisListType.X
OP = mybir.AluOpType
AF = mybir.ActivationFunctionType


def _emit_layernorm(nc, pool, x, y, gamma_bc, beta_bc, eps_t):
    """y = LN(x) * gamma + beta for one [P, D] token-major tile.

    Row sums via ACT accum_out (y doubles as the ACT scratch output);
    per-partition stats and the gamma/beta elementwise tail on DVE.
    """
    st = pool.tile([P, 8], FP32, tag="lnst", bufs=3, name="lnst")
    nc.scalar.activation(y, x, AF.Copy, accum_out=st[:, 0:1])       # sum x
    nc.scalar.activation(y, x, AF.Square, accum_out=st[:, 1:2])     # sum x^2
    nc.vector.tensor_scalar_mul(st[:, 2:3], st[:, 0:1], 1.0 / D)    # mu
    nc.vector.tensor_tensor(st[:, 3:4], st[:, 2:3], st[:, 2:3], OP.mult)
    nc.vector.tensor_scalar_mul(st[:, 4:5], st[:, 1:2], 1.0 / D)    # E[x^2]
    nc.vector.tensor_tensor(st[:, 4:5], st[:, 4:5], st[:, 3:4], OP.subtract)
    nc.scalar.activation(st[:, 5:6], st[:, 4:5], AF.Sqrt, bias=eps_t)
    nc.vector.reciprocal(st[:, 6:7], st[:, 5:6])                    # rstd
    nc.vector.tensor_tensor(st[:, 7:8], st[:, 2:3], st[:, 6:7], OP.mult)
    nc.vector.tensor_scalar_mul(st[:, 7:8], st[:, 7:8], -1.0)       # -mu*rstd
    # xn = x*rstd - mu*rstd on ACT (per-partition scale/bias APs)
    nc.scalar.activation(y, x, AF.Identity, bias=st[:, 7:8], scale=st[:, 6:7])
    nc.vector.tensor_tensor(y, y, gamma_bc, OP.mult)
    nc.vector.tensor_tensor(y, y, beta_bc, OP.add)


def build_bass():
    nc = bacc.Bacc("TRN2", target_bir_lowering=False, debug=False)

    q_d = nc.dram_tensor("q", [L, D], FP32, kind="ExternalInput")
    k_d = nc.dram_tensor("k", [L, D], FP32, kind="ExternalInput")
    v_d = nc.dram_tensor("v", [L, D], FP32, kind="ExternalInput")
    wq_d = nc.dram_tensor("wq", [D, D], FP32R, kind="ExternalInput")
    wk_d = nc.dram_tensor("wk", [D, D], FP32R, kind="ExternalInput")
    wv_d = nc.dram_tensor("wv", [D, D], FP32R, kind="ExternalInput")
    wo_d = nc.dram_tensor("wo", [D, D], BF16, kind="ExternalInput")
    gb_d = nc.dram_tensor("gb", [P, D], FP32, kind="ExternalInput")
    bb_d = nc.dram_tensor("bb", [P, D], FP32, kind="ExternalInput")
    id_d = nc.dram_tensor("ident", [P, P], FP32, kind="ExternalInput")
    on_d = nc.dram_tensor("ones1", [1, HD], FP32R, kind="ExternalInput")
    ep_d = nc.dram_tensor("epsc", [P, 1], FP32, kind="ExternalInput")
    vo_d = nc.dram_tensor("vone", [P, H * E], BF16, kind="ExternalInput")
    zz_d = nc.dram_tensor("zz", [HD, L], FP32R, kind="ExternalInput")
    out_d = nc.dram_tensor("out", [L, D], FP32, kind="ExternalOutput")

    with tile.TileContext(nc) as tc:
        with (
            tc.tile_pool(name="const", bufs=1) as cpool,
            tc.tile_pool(name="otp", bufs=1) as otp,
        ):
            gamma_bc = cpool.tile([P, D], FP32, name="gamma_bc")
            beta_bc = cpool.tile([P, D], FP32, name="beta_bc")
            ident = cpool.tile([P, P], FP32, name="ident")
            eps_t = cpool.tile([P, 1], FP32, name="eps_t")
            ones1 = cpool.tile([1, HD], FP32R, name="ones1")
            nc.sync.dma_start(gamma_bc, gb_d[:])
            nc.sync.dma_start(beta_bc, bb_d[:])
            nc.sync.dma_start(ident, id_d[:])
            nc.sync.dma_start(eps_t, ep_d[:])
            nc.sync.dma_start(ones1, on_d[:])

            def load_w(pool, dram, dtype):
                tiles = []
                for i in range(NC):
                    wt = pool.tile([P, D], dtype, tag="w", bufs=8,
                                   name=f"w{i}")
                    nc.sync.dma_start(wt, dram[i * P:(i + 1) * P, :])
                    tiles.append(wt)
                return tiles

            OT = [otp.tile([P, L], BF16, tag="ot", bufs=8, name=f"ot{j}")
                  for j in range(H // 2)]

            with tc.tile_pool(name="qkv", bufs=1) as qkv:
                QT = qkv.tile([P, NC, L], FP32R, tag="QT", name="QT")
                KT = qkv.tile([P, NC, L], FP32R, tag="KT", name="KT")
                Vaug = qkv.tile([P, NT, H * E], BF16, tag="Vaug", name="Vaug")

                with (
                    tc.tile_pool(name="actT", bufs=1) as atp,
                    tc.tile_pool(name="ps1", bufs=6, space="PSUM") as ps1,
                ):

                    def load_transpose(src_d, do_ln):
                        """Load token-major [L, D], optionally LN, and build
                        8 feature-major [P, L] chunks."""
                        chunks = [atp.tile([P, L], FP32R, tag="actT", bufs=8,
                                           name=f"tchunk{c}")
                                  for c in range(NC)]
                        for t in range(NT):
                            x = atp.tile([P, D], FP32, tag="in", bufs=3,
                                         name="x_in")
                            nc.sync.dma_start(x, src_d[t * P:(t + 1) * P, :])
                            if do_ln:
                                y = atp.tile([P, D], FP32, tag="qn", bufs=3,
                                             name="x_ln")
                                _emit_layernorm(nc, atp, x, y, gamma_bc,
                                                beta_bc, eps_t)
                                x = y
                            for c in range(NC):
                                pt = ps1.tile([P, P], FP32, tag="mm",
                                              name="ps_tr")
                                nc.tensor.transpose(
                                    pt, x[:, c * P:(c + 1) * P], ident)
                                nc.vector.tensor_copy(
                                    chunks[c][:, t * P:(t + 1) * P], pt)
                        return chunks

                    def proj_featmajor(w_tiles, act_chunks, OUTT):
                        """OUTT[:, m, :] = (W^T @ actT) feature-major."""
                        for m in range(NC):
                            for n in range(NQ):
                                ps = ps1.tile([P, QH], FP32, tag="mm",
                                              name="ps_pj")
                                for i in range(NC):
                                    nc.tensor.matmul(
                                        ps,
                                        w_tiles[i][:, m * P:(m + 1) * P],
                                        act_chunks[i][:, n * QH:(n + 1) * QH],
                                        start=(i == 0), stop=(i == NC - 1),
                                    )
                                nc.vector.tensor_copy(
                                    OUTT[:, m, n * QH:(n + 1) * QH], ps)

                    # --- k -> kT -> KT ---
                    kT = load_transpose(k_d, do_ln=False)
                    wk_t = load_w(atp, wk_d, FP32R)
                    proj_featmajor(wk_t, kT, KT)

                    # --- q -> LN -> qnT -> QT ---
                    qnT = load_transpose(q_d, do_ln=True)
                    wq_t = load_w(atp, wq_d, FP32R)
                    proj_featmajor(wq_t, qnT, QT)

                    # --- v -> vT -> V (token-major) -> Vaug ---
                    for t in range(NT):
                        nc.sync.dma_start(Vaug[:, t, :], vo_d[:])
                    vT = load_transpose(v_d, do_ln=False)
                    wv_t = load_w(atp, wv_d, FP32R)
                    for t in range(NT):
                        for n in range(NQ):
                            ps = ps1.tile([P, QH], FP32, tag="mm",
                                          name="ps_v")
                            for i in range(NC):
                                nc.tensor.matmul(
                                    ps,
                                    vT[i][:, t * P:(t + 1) * P],
                                    wv_t[i][:, n * QH:(n + 1) * QH],
                                    start=(i == 0), stop=(i == NC - 1),
                                )
                            dst = Vaug[:, t, n * 8 * E:(n + 1) * 8 * E]
                            dst = dst.rearrange("p (h e) -> p h e", e=E)
                            nc.vector.tensor_copy(
                                dst[:, :, 0:HD],
                                ps.rearrange("p (h d) -> p h d", d=HD))

                # ---------------- attention ----------------
                with (
                    tc.tile_pool(name="att", bufs=1) as att,
                    tc.tile_pool(name="ps2", bufs=1, space="PSUM") as ps2,
                ):
                    # zero-padded K^T copies: full-128 contraction keeps the
                    # PE array fully active (S matmuls shaped like the
                    # projection matmuls -> pipelined + HAM-warm)
                    KTza = att.tile([P, NC, L], FP32R, tag="ktz", bufs=2,
                                    name="KTza")
                    KTzb = att.tile([P, NC, L], FP32R, tag="ktz", bufs=2,
                                    name="KTzb")
                    for c in range(NC):
                        nc.sync.dma_start(KTza[HD:P, c, :], zz_d[:])
                        nc.sync.dma_start(KTzb[0:HD, c, :], zz_d[:])
                        nc.vector.tensor_copy(KTza[0:HD, c, :],
                                              KT[0:HD, c, :])
                        nc.vector.tensor_copy(KTzb[HD:P, c, :],
                                              KT[HD:P, c, :])
                    for j in range(H // 2):
                        ha, hb = 2 * j, 2 * j + 1
                        for n in range(NQ):
                            qs = slice(n * QH, (n + 1) * QH)
                            PTa = att.tile([P, NC, QH], BF16, tag="pt",
                                           bufs=3, name="pta")
                            PTb = att.tile([P, NC, QH], BF16, tag="pt",
                                           bufs=3, name="ptb")
                            for i in range(NC):
                                ks = slice(i * P, (i + 1) * P)
                                psa = ps2.tile([P, QH], FP32, tag="s",
                                               bufs=5, name="psa")
                                psb = ps2.tile([P, QH], FP32, tag="s",
                                               bufs=5, name="psb")
                                nc.tensor.matmul(
                                    psa, KTza[:, j, ks], QT[:, j, qs],
                                    start=True, stop=True)
                                nc.tensor.matmul(
                                    psb, KTzb[:, j, ks], QT[:, j, qs],
                                    start=True, stop=True)
                                nc.scalar.activation(PTa[:, i, :], psa, AF.Exp)
                                nc.scalar.activation(PTb[:, i, :], psb, AF.Exp)
                            poa = ps2.tile([E, QH], FP32, tag="o", bufs=2,
                                           name="poa")
                            pob = ps2.tile([E, QH], FP32, tag="o", bufs=2,
                                           name="pob")
                            for i in range(NC):
                                nc.tensor.matmul(
                                    poa, Vaug[:, i, ha * E:(ha + 1) * E],
                                    PTa[:, i, :],
                                    start=(i == 0), stop=(i == NC - 1))
                                nc.tensor.matmul(
                                    pob, Vaug[:, i, hb * E:(hb + 1) * E],
                                    PTb[:, i, :],
                                    start=(i == 0), stop=(i == NC - 1))
                            for sub, po in ((0, poa), (1, pob)):
                                o_tmp = att.tile([E, QH], FP32, tag="otmp",
                                                 bufs=2, name="o_tmp")
                                nc.vector.tensor_copy(o_tmp, po)
                                rin = att.tile([1, QH], FP32, tag="rin",
                                               bufs=2, name="rin")
                                nc.sync.dma_start(rin, o_tmp[HD:E, :])
                                rec = att.tile([1, QH], FP32R, tag="rec",
                                               bufs=2, name="rec")
                                c = RECIP_APPROX_FAST_CONSTS
                                nc.vector._custom_dve(
                                    RECIPROCAL_APPROX_FAST, out=rec, in0=rin,
                                    s0=c["s0"], s1=c["s1"], imm2=c["imm2"])
                                pbc = ps2.tile([HD, QH], FP32, tag="bc",
                                               bufs=1, name="pbc")
                                nc.tensor.matmul(pbc, ones1, rec,
                                                 start=True, stop=True)
                                if sub == 0:
                                    nc.vector.tensor_tensor(
                                        OT[j][0:HD, qs], o_tmp[0:HD, :],
                                        pbc, OP.mult)
                                else:
                                    oo = att.tile([HD, QH], BF16, tag="oo",
                                                  bufs=2, name="oo")
                                    nc.vector.tensor_tensor(
                                        oo, o_tmp[0:HD, :], pbc, OP.mult)
                                    nc.sync.dma_start(OT[j][HD:P, qs], oo)

            # ---------------- output projection + transpose + LN ---------
            with (
                tc.tile_pool(name="fin", bufs=1) as fin,
                tc.tile_pool(name="ps3", bufs=1, space="PSUM") as ps3,
            ):
                wo_t = load_w(fin, wo_d, BF16)
                res = []
                for t in range(NT):
                    rt = fin.tile([P, D], FP32, tag="res", bufs=8,
                                  name=f"res{t}")
                    nc.sync.dma_start(rt, q_d[t * P:(t + 1) * P, :])
                    res.append(rt)
                out_acc = [fin.tile([P, D], FP32, tag="oacc", bufs=8,
                                    name=f"oacc{t}")
                           for t in range(NT)]

                def emit_tr(m, pj):
                    """Transpose projT_m blocks into out_acc, fusing the
                    residual add."""
                    for t in range(NT):
                        pt = ps3.tile([P, P], FP32, tag="tr", bufs=4,
                                      name="ps_tr2")
                        nc.tensor.transpose(
                            pt, pj[:, t * P:(t + 1) * P], ident)
                        ms = slice(m * P, (m + 1) * P)
                        nc.vector.tensor_tensor(
                            out_acc[t][:, ms], pt, res[t][:, ms], OP.add)

                prev = None
                for m in range(NC):
                    pj = fin.tile([P, L], FP32, tag="pjt", bufs=3,
                                  name="pjt")
                    for n in range(NQ):
                        ps = ps3.tile([P, QH], FP32, tag="mm", bufs=4,
                                      name="ps_w")
                        for jj in range(NC):
                            nc.tensor.matmul(
                                ps,
                                wo_t[jj][:, m * P:(m + 1) * P],
                                OT[jj][:, n * QH:(n + 1) * QH],
                                start=(jj == 0), stop=(jj == NC - 1),
                            )
                        nc.vector.tensor_copy(pj[:, n * QH:(n + 1) * QH], ps)
                    if prev is not None:
                        emit_tr(m - 1, prev)
                    prev = pj
                emit_tr(NC - 1, prev)

                for t in range(NT):
                    u = out_acc[t]
                    y = fin.tile([P, D], FP32, tag="y", bufs=2, name="y")
                    _emit_layernorm(nc, fin, u, y, gamma_bc, beta_bc, eps_t)
                    nc.sync.dma_start(out_d[t * P:(t + 1) * P, :], y)

    nc.compile()
    return nc


_CACHE = {}


def _get_nc():
    if "nc" not in _CACHE:
        _CACHE["nc"] = build_bass()
    return _CACHE["nc"]


def make_in_maps(q, k, v, Wq, Wk, Wv, Wo, gamma, beta):
    q = np.asarray(q, np.float32)
    k = np.asarray(k, np.float32)
    v = np.asarray(v, np.float32)
    # fold the 1/sqrt(dk) attention scale into Wq (0.125 is exact in fp32)
    wq = (np.asarray(Wq, np.float32) * 0.125).astype(np.float32)
    wk = np.ascontiguousarray(np.asarray(Wk, np.float32))
    wv = np.ascontiguousarray(np.asarray(Wv, np.float32))
    wo = np.asarray(Wo, np.float32).astype(ml_dtypes.bfloat16)
    gb = np.ascontiguousarray(
        np.tile(np.asarray(gamma, np.float32)[None, :], (P, 1)))
    bb = np.ascontiguousarray(
        np.tile(np.asarray(beta, np.float32)[None, :], (P, 1)))
    ident = np.eye(P, dtype=np.float32)
    ones1 = np.ones((1, HD), np.float32)
    epsc = np.full((P, 1), EPS, np.float32)
    vone = np.ones((P, H * E), ml_dtypes.bfloat16)
    zz = np.zeros((HD, L), np.float32)
    B = q.shape[0]
    return [
        {
            "q": np.ascontiguousarray(q[b]),
            "k": np.ascontiguousarray(k[b]),
            "v": np.ascontiguousarray(v[b]),
            "wq": wq, "wk": wk, "wv": wv, "wo": wo,
            "gb": gb, "bb": bb, "ident": ident, "ones1": ones1,
            "epsc": epsc, "vone": vone, "zz": zz,
        }
        for b in range(B)
    ]


def kernel(q, k, v, Wq, Wk, Wv, Wo, gamma, beta, trace=False):
    from concourse.bass_utils import run_bass_kernel_spmd

    nc = _get_nc()
    in_maps = make_in_maps(q, k, v, Wq, Wk, Wv, Wo, gamma, beta)
    res = run_bass_kernel_spmd(nc, in_maps, core_ids=list(range(len(in_maps))),
                               trace=trace)
    out = np.stack([r["out"] for r in res.results], axis=0)
    if trace:
        return out, res
    return out


# revision 36
# speedup vs baseline: 1.1706x; 1.0018x over previous
"""Trainium2 Bass kernel: pre-LN multi-head attention block (B=8, L=1024,
D=1024, H=16, dk=dv=64), data-parallel over batch across 8 NeuronCores.

Per core (one batch element):
  qn   = LN(q) ; QT = (Wq/8)^T-proj feature-major ; KT likewise ; V token-major
  S^T  = K_h Q_h^T per head (feature-major, softmax dim on partitions,
         head pairs row-packed onto PE row groups 0-63 / 64-127)
  P^T  = exp(S^T)               (no max-subtraction needed: |S| <~ 7)
  O^T  = V_aug^T P^T            (ones column in V_aug -> sumexp row for free)
  O    = O^T / sumexp           (approx-recip bcast via 1-row PE matmul)
  out  = LN(O @ Wo + q)
"""

import numpy as np
import ml_dtypes

import concourse.bass as bass
import concourse.mybir as mybir
import concourse.tile as tile
from concourse import bacc
from concourse.dve_ops import RECIP_APPROX_FAST_CONSTS, RECIPROCAL_APPROX_FAST

P = 128
L = 1024          # tokens per batch element
D = 1024          # model dim
H = 16            # heads
HD = 64           # head dim
E = HD + 1        # head dim + sumexp column
NC = D // P       # 8 feature chunks
NT = L // P       # 8 token chunks
NQ = 2            # halves of the q/free dimension
QH = 512
EPS = 1e-6

FP32 = mybir.dt.float32
BF16 = mybir.dt.bfloat16
FP32R = mybir.dt.float32r
AX = mybir.AxisListType.X
OP = mybir.AluOpType
AF = mybir.ActivationFunctionType


def _emit_layernorm(nc, pool, x, y, gamma_bc, beta_bc, eps_t):
    """y = LN(x) * gamma + beta for one [P, D] token-major tile.

    Row sums via ACT accum_out (y doubles as the ACT scratch output);
    per-partition stats and the gamma/beta elementwise tail on DVE.
    """
    st = pool.tile([P, 8], FP32, tag="lnst", bufs=3, name="lnst")
    nc.scalar.activation(y, x, AF.Copy, accum_out=st[:, 0:1])       # sum x
    nc.scalar.activation(y, x, AF.Square, accum_out=st[:, 1:2])     # sum x^2
    nc.vector.tensor_scalar_mul(st[:, 2:3], st[:, 0:1], 1.0 / D)    # mu
    nc.vector.tensor_tensor(st[:, 3:4], st[:, 2:3], st[:, 2:3], OP.mult)
    nc.vector.tensor_scalar_mul(st[:, 4:5], st[:, 1:2], 1.0 / D)    # E[x^2]
    nc.vector.tensor_tensor(st[:, 4:5], st[:, 4:5], st[:, 3:4], OP.subtract)
    nc.scalar.activation(st[:, 5:6], st[:, 4:5], AF.Sqrt, bias=eps_t)
    nc.vector.reciprocal(st[:, 6:7], st[:, 5:6])                    # rstd
    nc.vector.tensor_tensor(st[:, 7:8], st[:, 2:3], st[:, 6:7], OP.mult)
    nc.vector.tensor_scalar_mul(st[:, 7:8], st[:, 7:8], -1.0)       # -mu*rstd
    # xn = x*rstd - mu*rstd on ACT (per-partition scale/bias APs)
    nc.scalar.activation(y, x, AF.Identity, bias=st[:, 7:8], scale=st[:, 6:7])
    nc.vector.tensor_tensor(y, y, gamma_bc, OP.mult)
    nc.vector.tensor_tensor(y, y, beta_bc, OP.add)


def build_bass():
    nc = bacc.Bacc("TRN2", target_bir_lowering=False, debug=False)

    q_d = nc.dram_tensor("q", [L, D], FP32, kind="ExternalInput")
    k_d = nc.dram_tensor("k", [L, D], FP32, kind="ExternalInput")
    v_d = nc.dram_tensor("v", [L, D], FP32, kind="ExternalInput")
    wq_d = nc.dram_tensor("wq", [D, D], FP32R, kind="ExternalInput")
    wk_d = nc.dram_tensor("wk", [D, D], FP32R, kind="ExternalInput")
    wv_d = nc.dram_tensor("wv", [D, D], FP32R, kind="ExternalInput")
    wo_d = nc.dram_tensor("wo", [D, D], BF16, kind="ExternalInput")
    gb_d = nc.dram_tensor("gb", [P, D], FP32, kind="ExternalInput")
    bb_d = nc.dram_tensor("bb", [P, D], FP32, kind="ExternalInput")
    id_d = nc.dram_tensor("ident", [P, P], FP32, kind="ExternalInput")
    on_d = nc.dram_tensor("ones1", [1, HD], FP32R, kind="ExternalInput")
    ep_d = nc.dram_tensor("epsc", [P, 1], FP32, kind="ExternalInput")
    vo_d = nc.dram_tensor("vone", [P, H * E], BF16, kind="ExternalInput")
    zz_d = nc.dram_tensor("zz", [HD, L], FP32R, kind="ExternalInput")
    out_d = nc.dram_tensor("out", [L, D], FP32, kind="ExternalOutput")

    with tile.TileContext(nc) as tc:
        with (
            tc.tile_pool(name="const", bufs=1) as cpool,
            tc.tile_pool(name="otp", bufs=1) as otp,
        ):
            gamma_bc = cpool.tile([P, D], FP32, name="gamma_bc")
            beta_bc = cpool.tile([P, D], FP32, name="beta_bc")
            ident = cpool.tile([P, P], FP32, name="ident")
            eps_t = cpool.tile([P, 1], FP32, name="eps_t")
            ones1 = cpool.tile([1, HD], FP32R, name="ones1")
            nc.sync.dma_start(gamma_bc, gb_d[:])
            nc.sync.dma_start(beta_bc, bb_d[:])
            nc.sync.dma_start(ident, id_d[:])
            nc.sync.dma_start(eps_t, ep_d[:])
            nc.sync.dma_start(ones1, on_d[:])

            def load_w(pool, dram, dtype):
                tiles = []
                for i in range(NC):
                    wt = pool.tile([P, D], dtype, tag="w", bufs=8,
                                   name=f"w{i}")
                    nc.sync.dma_start(wt, dram[i * P:(i + 1) * P, :])
                    tiles.append(wt)
                return tiles

            OT = [otp.tile([P, L], BF16, tag="ot", bufs=8, name=f"ot{j}")
                  for j in range(H // 2)]

            with tc.tile_pool(name="qkv", bufs=1) as qkv:
                QT = qkv.tile([P, NC, L], FP32R, tag="QT", name="QT")
                KT = qkv.tile([P, NC, L], FP32R, tag="KT", name="KT")
                Vaug = qkv.tile([P, NT, H * E], BF16, tag="Vaug", name="Vaug")

                with (
                    tc.tile_pool(name="actT", bufs=1) as atp,
                    tc.tile_pool(name="ps1", bufs=6, space="PSUM") as ps1,
                ):

                    def load_transpose(src_d, do_ln):
                        """Load token-major [L, D], optionally LN, and build
                        8 feature-major [P, L] chunks."""
                        chunks = [atp.tile([P, L], FP32R, tag="actT", bufs=8,
                                           name=f"tchunk{c}")
                                  for c in range(NC)]
                        for t in range(NT):
                            x = atp.tile([P, D], FP32, tag="in", bufs=3,
                                         name="x_in")
                            nc.sync.dma_start(x, src_d[t * P:(t + 1) * P, :])
                            if do_ln:
                                y = atp.tile([P, D], FP32, tag="qn", bufs=6,
                                             name="x_ln")
                                _emit_layernorm(nc, atp, x, y, gamma_bc,
                                                beta_bc, eps_t)
                                x = y
                            for c in range(NC):
                                pt = ps1.tile([P, P], FP32, tag="mm",
                                              name="ps_tr")
                                nc.tensor.transpose(
                                    pt, x[:, c * P:(c + 1) * P], ident)
                                nc.vector.tensor_copy(
                                    chunks[c][:, t * P:(t + 1) * P], pt)
                        return chunks

                    def proj_featmajor(w_tiles, act_chunks, OUTT):
                        """OUTT[:, m, :] = (W^T @ actT) feature-major."""
                        for m in range(NC):
                            for n in range(NQ):
                                ps = ps1.tile([P, QH], FP32, tag="mm",
                                              name="ps_pj")
                                for i in range(NC):
                                    nc.tensor.matmul(
                                        ps,
                                        w_tiles[i][:, m * P:(m + 1) * P],
                                        act_chunks[i][:, n * QH:(n + 1) * QH],
                                        start=(i == 0), stop=(i == NC - 1),
                                    )
                                nc.vector.tensor_copy(
                                    OUTT[:, m, n * QH:(n + 1) * QH], ps)

                    # --- k -> kT -> KT ---
                    kT = load_transpose(k_d, do_ln=False)
                    wk_t = load_w(atp, wk_d, FP32R)
                    proj_featmajor(wk_t, kT, KT)

                    # --- q -> LN -> qnT -> QT ---
                    qnT = load_transpose(q_d, do_ln=True)
                    wq_t = load_w(atp, wq_d, FP32R)
                    proj_featmajor(wq_t, qnT, QT)

                    # --- v -> vT -> V (token-major) -> Vaug ---
                    for t in range(NT):
                        nc.sync.dma_start(Vaug[:, t, :], vo_d[:])
                    vT = load_transpose(v_d, do_ln=False)
                    wv_t = load_w(atp, wv_d, FP32R)
                    for t in range(NT):
                        for n in range(NQ):
                            ps = ps1.tile([P, QH], FP32, tag="mm",
                                          name="ps_v")
                            for i in range(NC):
                                nc.tensor.matmul(
                                    ps,
                                    vT[i][:, t * P:(t + 1) * P],
                                    wv_t[i][:, n * QH:(n + 1) * QH],
                                    start=(i == 0), stop=(i == NC - 1),
                                )
                            dst = Vaug[:, t, n * 8 * E:(n + 1) * 8 * E]
                            dst = dst.rearrange("p (h e) -> p h e", e=E)
                            nc.vector.tensor_copy(
                                dst[:, :, 0:HD],
                                ps.rearrange("p (h d) -> p h d", d=HD))

                # ---------------- attention ----------------
                with (
                    tc.tile_pool(name="att", bufs=1) as att,
                    tc.tile_pool(name="ps2", bufs=1, space="PSUM") as ps2,
                ):
                    # zero-padded K^T copies: full-128 contraction keeps the
                    # PE array fully active (S matmuls shaped like the
                    # projection matmuls -> pipelined + HAM-warm)
                    KTza = att.tile([P, NC, L], FP32R, tag="ktz", bufs=2,
                                    name="KTza")
                    KTzb = att.tile([P, NC, L], FP32R, tag="ktz", bufs=2,
                                    name="KTzb")
                    for c in range(NC):
                        nc.sync.dma_start(KTza[HD:P, c, :], zz_d[:])
                        nc.sync.dma_start(KTzb[0:HD, c, :], zz_d[:])
                        nc.vector.tensor_copy(KTza[0:HD, c, :],
                                              KT[0:HD, c, :])
                        nc.vector.tensor_copy(KTzb[HD:P, c, :],
                                              KT[HD:P, c, :])
                    for j in range(H // 2):
                        ha, hb = 2 * j, 2 * j + 1
                        for n in range(NQ):
                            qs = slice(n * QH, (n + 1) * QH)
                            PTa = att.tile([P, NC, QH], BF16, tag="pt",
                                           bufs=3, name="pta")
                            PTb = att.tile([P, NC, QH], BF16, tag="pt",
                                           bufs=3, name="ptb")
                            for i in range(NC):
                                ks = slice(i * P, (i + 1) * P)
                                psa = ps2.tile([P, QH], FP32, tag="s",
                                               bufs=5, name="psa")
                                psb = ps2.tile([P, QH], FP32, tag="s",
                                               bufs=5, name="psb")
                                nc.tensor.matmul(
                                    psa, KTza[:, j, ks], QT[:, j, qs],
                                    start=True, stop=True)
                                nc.tensor.matmul(
                                    psb, KTzb[:, j, ks], QT[:, j, qs],
                                    start=True, stop=True)
                                nc.scalar.activation(PTa[:, i, :], psa, AF.Exp)
                                nc.scalar.activation(PTb[:, i, :], psb, AF.Exp)
                            poa = ps2.tile([E, QH], FP32, tag="o", bufs=2,
                                           name="poa")
                            pob = ps2.tile([E, QH], FP32, tag="o", bufs=2,
                                           name="pob")
                            for i in range(NC):
                                nc.tensor.matmul(
                                    poa, Vaug[:, i, ha * E:(ha + 1) * E],
                                    PTa[:, i, :],
                                    start=(i == 0), stop=(i == NC - 1))
                                nc.tensor.matmul(
                                    pob, Vaug[:, i, hb * E:(hb + 1) * E],
                                    PTb[:, i, :],
                                    start=(i == 0), stop=(i == NC - 1))
                            for sub, po in ((0, poa), (1, pob)):
                                o_tmp = att.tile([E, QH], FP32, tag="otmp",
                                                 bufs=2, name="o_tmp")
                                nc.vector.tensor_copy(o_tmp, po)
                                rin = att.tile([1, QH], FP32, tag="rin",
                                               bufs=2, name="rin")
                                nc.sync.dma_start(rin, o_tmp[HD:E, :])
                                rec = att.tile([1, QH], FP32R, tag="rec",
                                               bufs=2, name="rec")
                                c = RECIP_APPROX_FAST_CONSTS
                                nc.vector._custom_dve(
                                    RECIPROCAL_APPROX_FAST, out=rec, in0=rin,
                                    s0=c["s0"], s1=c["s1"], imm2=c["imm2"])
                                pbc = ps2.tile([HD, QH], FP32, tag="bc",
                                               bufs=1, name="pbc")
                                nc.tensor.matmul(pbc, ones1, rec,
                                                 start=True, stop=True)
                                if sub == 0:
                                    nc.vector.tensor_tensor(
                                        OT[j][0:HD, qs], o_tmp[0:HD, :],
                                        pbc, OP.mult)
                                else:
                                    oo = att.tile([HD, QH], BF16, tag="oo",
                                                  bufs=2, name="oo")
                                    nc.vector.tensor_tensor(
                                        oo, o_tmp[0:HD, :], pbc, OP.mult)
                                    nc.sync.dma_start(OT[j][HD:P, qs], oo)

            # ---------------- output projection + transpose + LN ---------
            with (
                tc.tile_pool(name="fin", bufs=1) as fin,
                tc.tile_pool(name="ps3", bufs=1, space="PSUM") as ps3,
            ):
                wo_t = load_w(fin, wo_d, BF16)
                res = []
                for t in range(NT):
                    rt = fin.tile([P, D], FP32, tag="res", bufs=8,
                                  name=f"res{t}")
                    nc.sync.dma_start(rt, q_d[t * P:(t + 1) * P, :])
                    res.append(rt)
                out_acc = [fin.tile([P, D], FP32, tag="oacc", bufs=8,
                                    name=f"oacc{t}")
                           for t in range(NT)]

                def emit_tr(m, pj):
                    """Transpose projT_m blocks into out_acc, fusing the
                    residual add."""
                    for t in range(NT):
                        pt = ps3.tile([P, P], FP32, tag="tr", bufs=4,
                                      name="ps_tr2")
                        nc.tensor.transpose(
                            pt, pj[:, t * P:(t + 1) * P], ident)
                        ms = slice(m * P, (m + 1) * P)
                        nc.vector.tensor_tensor(
                            out_acc[t][:, ms], pt, res[t][:, ms], OP.add)

                prev = None
                for m in range(NC):
                    pj = fin.tile([P, L], FP32, tag="pjt", bufs=3,
                                  name="pjt")
                    for n in range(NQ):
                        ps = ps3.tile([P, QH], FP32, tag="mm", bufs=4,
                                      name="ps_w")
                        for jj in range(NC):
                            nc.tensor.matmul(
                                ps,
                                wo_t[jj][:, m * P:(m + 1) * P],
                                OT[jj][:, n * QH:(n + 1) * QH],
                                start=(jj == 0), stop=(jj == NC - 1),
                            )
                        nc.vector.tensor_copy(pj[:, n * QH:(n + 1) * QH], ps)
                    if prev is not None:
                        emit_tr(m - 1, prev)
                    prev = pj
                emit_tr(NC - 1, prev)

                for t in range(NT):
                    u = out_acc[t]
                    y = fin.tile([P, D], FP32, tag="y", bufs=2, name="y")
                    _emit_layernorm(nc, fin, u, y, gamma_bc, beta_bc, eps_t)
                    nc.sync.dma_start(out_d[t * P:(t + 1) * P, :], y)

    nc.compile()
    return nc


_CACHE = {}


def _get_nc():
    if "nc" not in _CACHE:
        _CACHE["nc"] = build_bass()
    return _CACHE["nc"]


def make_in_maps(q, k, v, Wq, Wk, Wv, Wo, gamma, beta):
    q = np.asarray(q, np.float32)
    k = np.asarray(k, np.float32)
    v = np.asarray(v, np.float32)
    # fold the 1/sqrt(dk) attention scale into Wq (0.125 is exact in fp32)
    wq = (np.asarray(Wq, np.float32) * 0.125).astype(np.float32)
    wk = np.ascontiguousarray(np.asarray(Wk, np.float32))
    wv = np.ascontiguousarray(np.asarray(Wv, np.float32))
    wo = np.asarray(Wo, np.float32).astype(ml_dtypes.bfloat16)
    gb = np.ascontiguousarray(
        np.tile(np.asarray(gamma, np.float32)[None, :], (P, 1)))
    bb = np.ascontiguousarray(
        np.tile(np.asarray(beta, np.float32)[None, :], (P, 1)))
    ident = np.eye(P, dtype=np.float32)
    ones1 = np.ones((1, HD), np.float32)
    epsc = np.full((P, 1), EPS, np.float32)
    vone = np.ones((P, H * E), ml_dtypes.bfloat16)
    zz = np.zeros((HD, L), np.float32)
    B = q.shape[0]
    return [
        {
            "q": np.ascontiguousarray(q[b]),
            "k": np.ascontiguousarray(k[b]),
            "v": np.ascontiguousarray(v[b]),
            "wq": wq, "wk": wk, "wv": wv, "wo": wo,
            "gb": gb, "bb": bb, "ident": ident, "ones1": ones1,
            "epsc": epsc, "vone": vone, "zz": zz,
        }
        for b in range(B)
    ]


def kernel(q, k, v, Wq, Wk, Wv, Wo, gamma, beta, trace=False):
    from concourse.bass_utils import run_bass_kernel_spmd

    nc = _get_nc()
    in_maps = make_in_maps(q, k, v, Wq, Wk, Wv, Wo, gamma, beta)
    res = run_bass_kernel_spmd(nc, in_maps, core_ids=list(range(len(in_maps))),
                               trace=trace)
    out = np.stack([r["out"] for r in res.results], axis=0)
    if trace:
        return out, res
    return out


# revision 37
# speedup vs baseline: 1.1709x; 1.0002x over previous
"""Trainium2 Bass kernel: pre-LN multi-head attention block (B=8, L=1024,
D=1024, H=16, dk=dv=64), data-parallel over batch across 8 NeuronCores.

Per core (one batch element):
  qn   = LN(q) ; QT = (Wq/8)^T-proj feature-major ; KT likewise ; V token-major
  S^T  = K_h Q_h^T per head (feature-major, softmax dim on partitions,
         head pairs row-packed onto PE row groups 0-63 / 64-127)
  P^T  = exp(S^T)               (no max-subtraction needed: |S| <~ 7)
  O^T  = V_aug^T P^T            (ones column in V_aug -> sumexp row for free)
  O    = O^T / sumexp           (approx-recip bcast via 1-row PE matmul)
  out  = LN(O @ Wo + q)
"""

import numpy as np
import ml_dtypes

import concourse.bass as bass
import concourse.mybir as mybir
import concourse.tile as tile
from concourse import bacc
from concourse.dve_ops import RECIP_APPROX_FAST_CONSTS, RECIPROCAL_APPROX_FAST

P = 128
L = 1024          # tokens per batch element
D = 1024          # model dim
H = 16            # heads
HD = 64           # head dim
E = HD + 1        # head dim + sumexp column
NC = D // P       # 8 feature chunks
NT = L // P       # 8 token chunks
NQ = 2            # halves of the q/free dimension
QH = 512
EPS = 1e-6

FP32 = mybir.dt.float32
BF16 = mybir.dt.bfloat16
FP32R = mybir.dt.float32r
AX = mybir.AxisListType.X
OP = mybir.AluOpType
AF = mybir.ActivationFunctionType


def _emit_layernorm(nc, pool, x, y, gamma_bc, beta_bc, eps_t):
    """y = LN(x) * gamma + beta for one [P, D] token-major tile.

    Row sums via ACT accum_out (y doubles as the ACT scratch output);
    per-partition stats and the gamma/beta elementwise tail on DVE.
    """
    st = pool.tile([P, 8], FP32, tag="lnst", bufs=3, name="lnst")
    nc.scalar.activation(y, x, AF.Copy, accum_out=st[:, 0:1])       # sum x
    nc.scalar.activation(y, x, AF.Square, accum_out=st[:, 1:2])     # sum x^2
    nc.vector.tensor_scalar_mul(st[:, 2:3], st[:, 0:1], 1.0 / D)    # mu
    nc.vector.tensor_tensor(st[:, 3:4], st[:, 2:3], st[:, 2:3], OP.mult)
    nc.vector.tensor_scalar_mul(st[:, 4:5], st[:, 1:2], 1.0 / D)    # E[x^2]
    nc.vector.tensor_tensor(st[:, 4:5], st[:, 4:5], st[:, 3:4], OP.subtract)
    nc.scalar.activation(st[:, 5:6], st[:, 4:5], AF.Sqrt, bias=eps_t)
    nc.vector.reciprocal(st[:, 6:7], st[:, 5:6])                    # rstd
    nc.vector.tensor_tensor(st[:, 7:8], st[:, 2:3], st[:, 6:7], OP.mult)
    nc.vector.tensor_scalar_mul(st[:, 7:8], st[:, 7:8], -1.0)       # -mu*rstd
    # xn = x*rstd - mu*rstd on ACT (per-partition scale/bias APs)
    nc.scalar.activation(y, x, AF.Identity, bias=st[:, 7:8], scale=st[:, 6:7])
    nc.vector.tensor_tensor(y, y, gamma_bc, OP.mult)
    nc.vector.tensor_tensor(y, y, beta_bc, OP.add)


def build_bass():
    nc = bacc.Bacc("TRN2", target_bir_lowering=False, debug=False)

    q_d = nc.dram_tensor("q", [L, D], FP32, kind="ExternalInput")
    k_d = nc.dram_tensor("k", [L, D], FP32, kind="ExternalInput")
    v_d = nc.dram_tensor("v", [L, D], FP32, kind="ExternalInput")
    wq_d = nc.dram_tensor("wq", [D, D], FP32R, kind="ExternalInput")
    wk_d = nc.dram_tensor("wk", [D, D], FP32R, kind="ExternalInput")
    wv_d = nc.dram_tensor("wv", [D, D], FP32R, kind="ExternalInput")
    wo_d = nc.dram_tensor("wo", [D, D], BF16, kind="ExternalInput")
    gb_d = nc.dram_tensor("gb", [P, D], FP32, kind="ExternalInput")
    bb_d = nc.dram_tensor("bb", [P, D], FP32, kind="ExternalInput")
    id_d = nc.dram_tensor("ident", [P, P], FP32, kind="ExternalInput")
    on_d = nc.dram_tensor("ones1", [1, HD], FP32R, kind="ExternalInput")
    ep_d = nc.dram_tensor("epsc", [P, 1], FP32, kind="ExternalInput")
    vo_d = nc.dram_tensor("vone", [P, H * E], BF16, kind="ExternalInput")
    zz_d = nc.dram_tensor("zz", [HD, L], FP32R, kind="ExternalInput")
    out_d = nc.dram_tensor("out", [L, D], FP32, kind="ExternalOutput")

    with tile.TileContext(nc) as tc:
        with (
            tc.tile_pool(name="const", bufs=1) as cpool,
            tc.tile_pool(name="otp", bufs=1) as otp,
        ):
            gamma_bc = cpool.tile([P, D], FP32, name="gamma_bc")
            beta_bc = cpool.tile([P, D], FP32, name="beta_bc")
            ident = cpool.tile([P, P], FP32, name="ident")
            eps_t = cpool.tile([P, 1], FP32, name="eps_t")
            ones1 = cpool.tile([1, HD], FP32R, name="ones1")
            nc.sync.dma_start(gamma_bc, gb_d[:])
            nc.sync.dma_start(beta_bc, bb_d[:])
            nc.sync.dma_start(ident, id_d[:])
            nc.sync.dma_start(eps_t, ep_d[:])
            nc.sync.dma_start(ones1, on_d[:])

            def load_w(pool, dram, dtype):
                tiles = []
                for i in range(NC):
                    wt = pool.tile([P, D], dtype, tag="w", bufs=8,
                                   name=f"w{i}")
                    nc.sync.dma_start(wt, dram[i * P:(i + 1) * P, :])
                    tiles.append(wt)
                return tiles

            OT = [otp.tile([P, L], BF16, tag="ot", bufs=8, name=f"ot{j}")
                  for j in range(H // 2)]

            with tc.tile_pool(name="qkv", bufs=1) as qkv:
                QT = qkv.tile([P, NC, L], FP32R, tag="QT", name="QT")
                KT = qkv.tile([P, NC, L], FP32R, tag="KT", name="KT")
                Vaug = qkv.tile([P, NT, H * E], BF16, tag="Vaug", name="Vaug")

                with (
                    tc.tile_pool(name="actT", bufs=1) as atp,
                    tc.tile_pool(name="ps1", bufs=6, space="PSUM") as ps1,
                ):

                    def load_transpose(src_d, do_ln):
                        """Load token-major [L, D], optionally LN, and build
                        8 feature-major [P, L] chunks."""
                        chunks = [atp.tile([P, L], FP32R, tag="actT", bufs=8,
                                           name=f"tchunk{c}")
                                  for c in range(NC)]
                        for t in range(NT):
                            x = atp.tile([P, D], FP32, tag="in", bufs=3,
                                         name="x_in")
                            nc.sync.dma_start(x, src_d[t * P:(t + 1) * P, :])
                            if do_ln:
                                y = atp.tile([P, D], FP32, tag="qn", bufs=3,
                                             name="x_ln")
                                _emit_layernorm(nc, atp, x, y, gamma_bc,
                                                beta_bc, eps_t)
                                x = y
                            for c in range(NC):
                                pt = ps1.tile([P, P], FP32, tag="mm",
                                              name="ps_tr")
                                nc.tensor.transpose(
                                    pt, x[:, c * P:(c + 1) * P], ident)
                                nc.vector.tensor_copy(
                                    chunks[c][:, t * P:(t + 1) * P], pt)
                        return chunks

                    def proj_featmajor(w_tiles, act_chunks, OUTT):
                        """OUTT[:, m, :] = (W^T @ actT) feature-major."""
                        for m in range(NC):
                            for n in range(NQ):
                                ps = ps1.tile([P, QH], FP32, tag="mm",
                                              name="ps_pj")
                                for i in range(NC):
                                    nc.tensor.matmul(
                                        ps,
                                        w_tiles[i][:, m * P:(m + 1) * P],
                                        act_chunks[i][:, n * QH:(n + 1) * QH],
                                        start=(i == 0), stop=(i == NC - 1),
                                    )
                                nc.vector.tensor_copy(
                                    OUTT[:, m, n * QH:(n + 1) * QH], ps)

                    # --- k -> kT -> KT ---
                    kT = load_transpose(k_d, do_ln=False)
                    wk_t = load_w(atp, wk_d, FP32R)
                    proj_featmajor(wk_t, kT, KT)

                    # --- q -> LN -> qnT -> QT ---
                    qnT = load_transpose(q_d, do_ln=True)
                    wq_t = load_w(atp, wq_d, FP32R)
                    proj_featmajor(wq_t, qnT, QT)

                    # --- v -> vT -> V (token-major) -> Vaug ---
                    for t in range(NT):
                        nc.sync.dma_start(Vaug[:, t, :], vo_d[:])
                    vT = load_transpose(v_d, do_ln=False)
                    wv_t = load_w(atp, wv_d, FP32R)
                    for t in range(NT):
                        for n in range(NQ):
                            ps = ps1.tile([P, QH], FP32, tag="mm",
                                          name="ps_v")
                            for i in range(NC):
                                nc.tensor.matmul(
                                    ps,
                                    vT[i][:, t * P:(t + 1) * P],
                                    wv_t[i][:, n * QH:(n + 1) * QH],
                                    start=(i == 0), stop=(i == NC - 1),
                                )
                            dst = Vaug[:, t, n * 8 * E:(n + 1) * 8 * E]
                            dst = dst.rearrange("p (h e) -> p h e", e=E)
                            nc.vector.tensor_copy(
                                dst[:, :, 0:HD],
                                ps.rearrange("p (h d) -> p h d", d=HD))

                # ---------------- attention ----------------
                with (
                    tc.tile_pool(name="att", bufs=1) as att,
                    tc.tile_pool(name="ps2", bufs=1, space="PSUM") as ps2,
                ):
                    # zero-padded K^T copies: full-128 contraction keeps the
                    # PE array fully active (S matmuls shaped like the
                    # projection matmuls -> pipelined + HAM-warm)
                    KTza = att.tile([P, NC, L], FP32R, tag="ktz", bufs=2,
                                    name="KTza")
                    KTzb = att.tile([P, NC, L], FP32R, tag="ktz", bufs=2,
                                    name="KTzb")
                    for c in range(NC):
                        nc.sync.dma_start(KTza[HD:P, c, :], zz_d[:])
                        nc.sync.dma_start(KTzb[0:HD, c, :], zz_d[:])
                        nc.vector.tensor_copy(KTza[0:HD, c, :],
                                              KT[0:HD, c, :])
                        nc.vector.tensor_copy(KTzb[HD:P, c, :],
                                              KT[HD:P, c, :])
                    for j in range(H // 2):
                        ha, hb = 2 * j, 2 * j + 1
                        for n in range(NQ):
                            qs = slice(n * QH, (n + 1) * QH)
                            PTa = att.tile([P, NC, QH], BF16, tag="pt",
                                           bufs=3, name="pta")
                            PTb = att.tile([P, NC, QH], BF16, tag="pt",
                                           bufs=3, name="ptb")
                            for i in range(NC):
                                ks = slice(i * P, (i + 1) * P)
                                psa = ps2.tile([P, QH], FP32, tag="s",
                                               bufs=5, name="psa")
                                psb = ps2.tile([P, QH], FP32, tag="s",
                                               bufs=5, name="psb")
                                nc.tensor.matmul(
                                    psa, KTza[:, j, ks], QT[:, j, qs],
                                    start=True, stop=True)
                                nc.tensor.matmul(
                                    psb, KTzb[:, j, ks], QT[:, j, qs],
                                    start=True, stop=True)
                                nc.scalar.activation(PTa[:, i, :], psa, AF.Exp)
                                nc.scalar.activation(PTb[:, i, :], psb, AF.Exp)
                            poa = ps2.tile([E, QH], FP32, tag="o", bufs=2,
                                           name="poa")
                            pob = ps2.tile([E, QH], FP32, tag="o", bufs=2,
                                           name="pob")
                            for i in range(NC):
                                nc.tensor.matmul(
                                    poa, Vaug[:, i, ha * E:(ha + 1) * E],
                                    PTa[:, i, :],
                                    start=(i == 0), stop=(i == NC - 1))
                                nc.tensor.matmul(
                                    pob, Vaug[:, i, hb * E:(hb + 1) * E],
                                    PTb[:, i, :],
                                    start=(i == 0), stop=(i == NC - 1))
                            for sub, po in ((0, poa), (1, pob)):
                                o_tmp = att.tile([E, QH], FP32, tag="otmp",
                                                 bufs=2, name="o_tmp")
                                nc.vector.tensor_copy(o_tmp, po)
                                rin = att.tile([1, QH], FP32, tag="rin",
                                               bufs=2, name="rin")
                                nc.sync.dma_start(rin, o_tmp[HD:E, :])
                                rec = att.tile([1, QH], FP32R, tag="rec",
                                               bufs=2, name="rec")
                                c = RECIP_APPROX_FAST_CONSTS
                                nc.vector._custom_dve(
                                    RECIPROCAL_APPROX_FAST, out=rec, in0=rin,
                                    s0=c["s0"], s1=c["s1"], imm2=c["imm2"])
                                pbc = ps2.tile([HD, QH], FP32, tag="bc",
                                               bufs=1, name="pbc")
                                nc.tensor.matmul(pbc, ones1, rec,
                                                 start=True, stop=True)
                                if sub == 0:
                                    nc.vector.tensor_tensor(
                                        OT[j][0:HD, qs], o_tmp[0:HD, :],
                                        pbc, OP.mult)
                                else:
                                    oo = att.tile([HD, QH], BF16, tag="oo",
                                                  bufs=2, name="oo")
                                    nc.vector.tensor_tensor(
                                        oo, o_tmp[0:HD, :], pbc, OP.mult)
                                    nc.sync.dma_start(OT[j][HD:P, qs], oo)

            # ---------------- output projection + transpose + LN ---------
            with (
                tc.tile_pool(name="fin", bufs=1) as fin,
                tc.tile_pool(name="ps3", bufs=1, space="PSUM") as ps3,
            ):
                wo_t = load_w(fin, wo_d, BF16)
                res = []
                for t in range(NT):
                    rt = fin.tile([P, D], FP32, tag="res", bufs=8,
                                  name=f"res{t}")
                    nc.sync.dma_start(rt, q_d[t * P:(t + 1) * P, :])
                    res.append(rt)
                out_acc = [fin.tile([P, D], FP32, tag="oacc", bufs=8,
                                    name=f"oacc{t}")
                           for t in range(NT)]

                def emit_tr(m, pj):
                    """Transpose projT_m blocks into out_acc, fusing the
                    residual add."""
                    for t in range(NT):
                        pt = ps3.tile([P, P], FP32, tag="tr", bufs=4,
                                      name="ps_tr2")
                        nc.tensor.transpose(
                            pt, pj[:, t * P:(t + 1) * P], ident)
                        ms = slice(m * P, (m + 1) * P)
                        nc.vector.tensor_tensor(
                            out_acc[t][:, ms], pt, res[t][:, ms], OP.add)

                prev = None
                for m in range(NC):
                    pj = fin.tile([P, L], FP32, tag="pjt", bufs=3,
                                  name="pjt")
                    for n in range(NQ):
                        ps = ps3.tile([P, QH], FP32, tag="mm", bufs=4,
                                      name="ps_w")
                        for jj in range(NC):
                            nc.tensor.matmul(
                                ps,
                                wo_t[jj][:, m * P:(m + 1) * P],
                                OT[jj][:, n * QH:(n + 1) * QH],
                                start=(jj == 0), stop=(jj == NC - 1),
                            )
                        nc.vector.tensor_copy(pj[:, n * QH:(n + 1) * QH], ps)
                    if prev is not None:
                        emit_tr(m - 1, prev)
                    prev = pj
                emit_tr(NC - 1, prev)

                for t in range(NT):
                    u = out_acc[t]
                    y = fin.tile([P, D], FP32, tag="y", bufs=2, name="y")
                    _emit_layernorm(nc, fin, u, y, gamma_bc, beta_bc, eps_t)
                    nc.sync.dma_start(out_d[t * P:(t + 1) * P, :], y)

    nc.compile()
    return nc


_CACHE = {}


def _get_nc():
    if "nc" not in _CACHE:
        _CACHE["nc"] = build_bass()
    return _CACHE["nc"]


def make_in_maps(q, k, v, Wq, Wk, Wv, Wo, gamma, beta):
    q = np.asarray(q, np.float32)
    k = np.asarray(k, np.float32)
    v = np.asarray(v, np.float32)
    # fold the 1/sqrt(dk) attention scale into Wq (0.125 is exact in fp32)
    wq = (np.asarray(Wq, np.float32) * 0.125).astype(np.float32)
    wk = np.ascontiguousarray(np.asarray(Wk, np.float32))
    wv = np.ascontiguousarray(np.asarray(Wv, np.float32))
    wo = np.asarray(Wo, np.float32).astype(ml_dtypes.bfloat16)
    gb = np.ascontiguousarray(
        np.tile(np.asarray(gamma, np.float32)[None, :], (P, 1)))
    bb = np.ascontiguousarray(
        np.tile(np.asarray(beta, np.float32)[None, :], (P, 1)))
    ident = np.eye(P, dtype=np.float32)
    ones1 = np.ones((1, HD), np.float32)
    epsc = np.full((P, 1), EPS, np.float32)
    vone = np.ones((P, H * E), ml_dtypes.bfloat16)
    zz = np.zeros((HD, L), np.float32)
    B = q.shape[0]
    return [
        {
            "q": np.ascontiguousarray(q[b]),
            "k": np.ascontiguousarray(k[b]),
            "v": np.ascontiguousarray(v[b]),
            "wq": wq, "wk": wk, "wv": wv, "wo": wo,
            "gb": gb, "bb": bb, "ident": ident, "ones1": ones1,
            "epsc": epsc, "vone": vone, "zz": zz,
        }
        for b in range(B)
    ]


def kernel(q, k, v, Wq, Wk, Wv, Wo, gamma, beta, trace=False):
    from concourse.bass_utils import run_bass_kernel_spmd

    nc = _get_nc()
    in_maps = make_in_maps(q, k, v, Wq, Wk, Wv, Wo, gamma, beta)
    res = run_bass_kernel_spmd(nc, in_maps, core_ids=list(range(len(in_maps))),
                               trace=trace)
    out = np.stack([r["out"] for r in res.results], axis=0)
    if trace:
        return out, res
    return out


# revision 39
# speedup vs baseline: 1.1905x; 1.0168x over previous
"""Trainium2 Bass kernel: pre-LN multi-head attention block (B=8, L=1024,
D=1024, H=16, dk=dv=64), data-parallel over batch across 8 NeuronCores.

Per core (one batch element):
  qn   = LN(q) ; QT = (Wq/8)^T-proj feature-major ; KT likewise ; V token-major
  S^T  = K_h Q_h^T per head (feature-major, softmax dim on partitions,
         head pairs row-packed onto PE row groups 0-63 / 64-127)
  P^T  = exp(S^T)               (no max-subtraction needed: |S| <~ 7)
  O^T  = V_aug^T P^T            (ones column in V_aug -> sumexp row for free)
  O    = O^T / sumexp           (approx-recip bcast via 1-row PE matmul)
  out  = LN(O @ Wo + q)
"""

import numpy as np
import ml_dtypes

import concourse.bass as bass
import concourse.mybir as mybir
import concourse.tile as tile
from concourse import bacc
from concourse.dve_ops import RECIP_APPROX_FAST_CONSTS, RECIPROCAL_APPROX_FAST

P = 128
L = 1024          # tokens per batch element
D = 1024          # model dim
H = 16            # heads
HD = 64           # head dim
E = HD + 1        # head dim + sumexp column
NC = D // P       # 8 feature chunks
NT = L // P       # 8 token chunks
NQ = 2            # halves of the q/free dimension
QH = 512
EPS = 1e-6

FP32 = mybir.dt.float32
BF16 = mybir.dt.bfloat16
FP32R = mybir.dt.float32r
AX = mybir.AxisListType.X
OP = mybir.AluOpType
AF = mybir.ActivationFunctionType


def _emit_layernorm(nc, pool, x, y, gamma_bc, beta_bc, eps_t):
    """y = LN(x) * gamma + beta for one [P, D] token-major tile.

    Row sums via ACT accum_out (y doubles as the ACT scratch output);
    per-partition stats and the gamma/beta elementwise tail on DVE.
    """
    st = pool.tile([P, 8], FP32, tag="lnst", bufs=3, name="lnst")
    nc.scalar.activation(y, x, AF.Copy, accum_out=st[:, 0:1])       # sum x
    nc.scalar.activation(y, x, AF.Square, accum_out=st[:, 1:2])     # sum x^2
    nc.vector.tensor_scalar_mul(st[:, 2:3], st[:, 0:1], 1.0 / D)    # mu
    nc.vector.tensor_tensor(st[:, 3:4], st[:, 2:3], st[:, 2:3], OP.mult)
    nc.vector.tensor_scalar_mul(st[:, 4:5], st[:, 1:2], 1.0 / D)    # E[x^2]
    nc.vector.tensor_tensor(st[:, 4:5], st[:, 4:5], st[:, 3:4], OP.subtract)
    nc.scalar.activation(st[:, 5:6], st[:, 4:5], AF.Sqrt, bias=eps_t)
    nc.vector.reciprocal(st[:, 6:7], st[:, 5:6])                    # rstd
    nc.vector.tensor_tensor(st[:, 7:8], st[:, 2:3], st[:, 6:7], OP.mult)
    nc.vector.tensor_scalar_mul(st[:, 7:8], st[:, 7:8], -1.0)       # -mu*rstd
    # xn = x*rstd - mu*rstd on ACT (per-partition scale/bias APs)
    nc.scalar.activation(y, x, AF.Identity, bias=st[:, 7:8], scale=st[:, 6:7])
    nc.vector.tensor_tensor(y, y, gamma_bc, OP.mult)
    nc.vector.tensor_tensor(y, y, beta_bc, OP.add)


def build_bass():
    nc = bacc.Bacc("TRN2", target_bir_lowering=False, debug=False)

    q_d = nc.dram_tensor("q", [L, D], FP32, kind="ExternalInput")
    k_d = nc.dram_tensor("k", [L, D], FP32, kind="ExternalInput")
    v_d = nc.dram_tensor("v", [L, D], FP32, kind="ExternalInput")
    wq_d = nc.dram_tensor("wq", [D, D], FP32R, kind="ExternalInput")
    wk_d = nc.dram_tensor("wk", [D, D], FP32R, kind="ExternalInput")
    wv_d = nc.dram_tensor("wv", [D, D], FP32R, kind="ExternalInput")
    wo_d = nc.dram_tensor("wo", [D, D], BF16, kind="ExternalInput")
    gb_d = nc.dram_tensor("gb", [P, D], FP32, kind="ExternalInput")
    bb_d = nc.dram_tensor("bb", [P, D], FP32, kind="ExternalInput")
    id_d = nc.dram_tensor("ident", [P, P], FP32, kind="ExternalInput")
    on_d = nc.dram_tensor("ones1", [1, HD], FP32R, kind="ExternalInput")
    ep_d = nc.dram_tensor("epsc", [P, 1], FP32, kind="ExternalInput")
    vo_d = nc.dram_tensor("vone", [P, H * E], BF16, kind="ExternalInput")
    zz_d = nc.dram_tensor("zz", [HD, L], FP32R, kind="ExternalInput")
    out_d = nc.dram_tensor("out", [L, D], FP32, kind="ExternalOutput")

    with tile.TileContext(nc) as tc:
        with (
            tc.tile_pool(name="const", bufs=1) as cpool,
            tc.tile_pool(name="otp", bufs=1) as otp,
        ):
            gamma_bc = cpool.tile([P, D], FP32, name="gamma_bc")
            beta_bc = cpool.tile([P, D], FP32, name="beta_bc")
            ident = cpool.tile([P, P], FP32, name="ident")
            eps_t = cpool.tile([P, 1], FP32, name="eps_t")
            ones1 = cpool.tile([1, HD], FP32R, name="ones1")
            nc.sync.dma_start(gamma_bc, gb_d[:])
            nc.sync.dma_start(beta_bc, bb_d[:])
            nc.sync.dma_start(ident, id_d[:])
            nc.sync.dma_start(eps_t, ep_d[:])
            nc.sync.dma_start(ones1, on_d[:])

            def load_w(pool, dram, dtype):
                tiles = []
                for i in range(NC):
                    wt = pool.tile([P, D], dtype, tag="w", bufs=8,
                                   name=f"w{i}")
                    nc.sync.dma_start(wt, dram[i * P:(i + 1) * P, :])
                    tiles.append(wt)
                return tiles

            OT = [otp.tile([P, L], BF16, tag="ot", bufs=8, name=f"ot{j}")
                  for j in range(H // 2)]

            with tc.tile_pool(name="qkv", bufs=1) as qkv:
                QT = qkv.tile([P, NC, L], FP32R, tag="QT", name="QT")
                KT = qkv.tile([P, NC, L], FP32R, tag="KT", name="KT")
                Vaug = qkv.tile([P, NT, H * E], BF16, tag="Vaug", name="Vaug")

                with (
                    tc.tile_pool(name="actT", bufs=1) as atp,
                    tc.tile_pool(name="ps1", bufs=6, space="PSUM") as ps1,
                ):

                    def load_transpose(src_d, do_ln):
                        """Load token-major [L, D], optionally LN, and build
                        8 feature-major [P, L] chunks."""
                        chunks = [atp.tile([P, L], FP32R, tag="actT", bufs=8,
                                           name=f"tchunk{c}")
                                  for c in range(NC)]
                        for t in range(NT):
                            x = atp.tile([P, D], FP32, tag="in", bufs=3,
                                         name="x_in")
                            nc.sync.dma_start(x, src_d[t * P:(t + 1) * P, :])
                            if do_ln:
                                y = atp.tile([P, D], FP32, tag="qn", bufs=3,
                                             name="x_ln")
                                _emit_layernorm(nc, atp, x, y, gamma_bc,
                                                beta_bc, eps_t)
                                x = y
                            for c in range(NC):
                                pt = ps1.tile([P, P], FP32, tag="mm",
                                              name="ps_tr")
                                nc.tensor.transpose(
                                    pt, x[:, c * P:(c + 1) * P], ident)
                                nc.vector.tensor_copy(
                                    chunks[c][:, t * P:(t + 1) * P], pt)
                        return chunks

                    def proj_featmajor(w_tiles, act_chunks, OUTT):
                        """OUTT[:, m, :] = (W^T @ actT) feature-major."""
                        for m in range(NC):
                            for n in range(NQ):
                                ps = ps1.tile([P, QH], FP32, tag="mm",
                                              name="ps_pj")
                                for i in range(NC):
                                    nc.tensor.matmul(
                                        ps,
                                        w_tiles[i][:, m * P:(m + 1) * P],
                                        act_chunks[i][:, n * QH:(n + 1) * QH],
                                        start=(i == 0), stop=(i == NC - 1),
                                    )
                                nc.vector.tensor_copy(
                                    OUTT[:, m, n * QH:(n + 1) * QH], ps)

                    # --- k -> kT -> KT ---
                    kT = load_transpose(k_d, do_ln=False)
                    wk_t = load_w(atp, wk_d, FP32R)
                    proj_featmajor(wk_t, kT, KT)

                    # --- q -> LN -> qnT -> QT ---
                    qnT = load_transpose(q_d, do_ln=True)
                    wq_t = load_w(atp, wq_d, FP32R)
                    proj_featmajor(wq_t, qnT, QT)

                    # --- v -> vT -> V (token-major) -> Vaug ---
                    for t in range(NT):
                        nc.sync.dma_start(Vaug[:, t, :], vo_d[:])
                    vT = load_transpose(v_d, do_ln=False)
                    wv_t = load_w(atp, wv_d, FP32R)
                    for t in range(NT):
                        for n in range(NQ):
                            ps = ps1.tile([P, QH], FP32, tag="mm",
                                          name="ps_v")
                            for i in range(NC):
                                nc.tensor.matmul(
                                    ps,
                                    vT[i][:, t * P:(t + 1) * P],
                                    wv_t[i][:, n * QH:(n + 1) * QH],
                                    start=(i == 0), stop=(i == NC - 1),
                                )
                            dst = Vaug[:, t, n * 8 * E:(n + 1) * 8 * E]
                            dst = dst.rearrange("p (h e) -> p h e", e=E)
                            nc.vector.tensor_copy(
                                dst[:, :, 0:HD],
                                ps.rearrange("p (h d) -> p h d", d=HD))

                # ---------------- attention ----------------
                with (
                    tc.tile_pool(name="att", bufs=1) as att,
                    tc.tile_pool(name="ps2", bufs=1, space="PSUM") as ps2,
                ):
                    # zero-padded K^T copies: full-128 contraction keeps the
                    # PE array fully active (S matmuls shaped like the
                    # projection matmuls -> pipelined + HAM-warm)
                    KTza = att.tile([P, NC, L], FP32R, tag="ktz", bufs=2,
                                    name="KTza")
                    KTzb = att.tile([P, NC, L], FP32R, tag="ktz", bufs=2,
                                    name="KTzb")
                    for c in range(NC):
                        nc.sync.dma_start(KTza[HD:P, c, :], zz_d[:])
                        nc.sync.dma_start(KTzb[0:HD, c, :], zz_d[:])
                        nc.vector.tensor_copy(KTza[0:HD, c, :],
                                              KT[0:HD, c, :])
                        nc.vector.tensor_copy(KTzb[HD:P, c, :],
                                              KT[HD:P, c, :])
                    for j in range(H // 2):
                        ha, hb = 2 * j, 2 * j + 1
                        for n in range(NQ):
                            qs = slice(n * QH, (n + 1) * QH)
                            PTa = att.tile([P, NC, QH], BF16, tag="pt",
                                           bufs=3, name="pta")
                            PTb = att.tile([P, NC, QH], BF16, tag="pt",
                                           bufs=3, name="ptb")
                            for i in range(NC):
                                ks = slice(i * P, (i + 1) * P)
                                psa = ps2.tile([P, QH], FP32, tag="s",
                                               bufs=5, name="psa")
                                psb = ps2.tile([P, QH], FP32, tag="s",
                                               bufs=5, name="psb")
                                nc.tensor.matmul(
                                    psa, KTza[:, j, ks], QT[:, j, qs],
                                    start=True, stop=True)
                                nc.tensor.matmul(
                                    psb, KTzb[:, j, ks], QT[:, j, qs],
                                    start=True, stop=True)
                                nc.scalar.activation(PTa[:, i, :], psa, AF.Exp)
                                nc.scalar.activation(PTb[:, i, :], psb, AF.Exp)
                            poa = ps2.tile([E, QH], FP32, tag="o", bufs=2,
                                           name="poa")
                            pob = ps2.tile([E, QH], FP32, tag="o", bufs=2,
                                           name="pob")
                            for i in range(NC):
                                nc.tensor.matmul(
                                    poa, Vaug[:, i, ha * E:(ha + 1) * E],
                                    PTa[:, i, :],
                                    start=(i == 0), stop=(i == NC - 1))
                                nc.tensor.matmul(
                                    pob, Vaug[:, i, hb * E:(hb + 1) * E],
                                    PTb[:, i, :],
                                    start=(i == 0), stop=(i == NC - 1))
                            for sub, po in ((0, poa), (1, pob)):
                                o_tmp = att.tile([E, QH], FP32, tag="otmp",
                                                 bufs=2, name="o_tmp")
                                nc.vector.tensor_copy(o_tmp, po)
                                rin = att.tile([1, QH], FP32, tag="rin",
                                               bufs=2, name="rin")
                                nc.sync.dma_start(rin, o_tmp[HD:E, :])
                                rec = att.tile([1, QH], FP32R, tag="rec",
                                               bufs=2, name="rec")
                                c = RECIP_APPROX_FAST_CONSTS
                                nc.vector._custom_dve(
                                    RECIPROCAL_APPROX_FAST, out=rec, in0=rin,
                                    s0=c["s0"], s1=c["s1"], imm2=c["imm2"])
                                pbc = ps2.tile([HD, QH], FP32, tag="bc",
                                               bufs=1, name="pbc")
                                nc.tensor.matmul(pbc, ones1, rec,
                                                 start=True, stop=True)
                                if sub == 0:
                                    nc.vector.tensor_tensor(
                                        OT[j][0:HD, qs], o_tmp[0:HD, :],
                                        pbc, OP.mult)
                                else:
                                    oo = att.tile([HD, QH], BF16, tag="oo",
                                                  bufs=2, name="oo")
                                    nc.vector.tensor_tensor(
                                        oo, o_tmp[0:HD, :], pbc, OP.mult)
                                    nc.sync.dma_start(OT[j][HD:P, qs], oo)

            # ---------------- output projection + transpose + LN ---------
            with (
                tc.tile_pool(name="fin", bufs=1) as fin,
                tc.tile_pool(name="ps3", bufs=1, space="PSUM") as ps3,
            ):
                wo_t = load_w(fin, wo_d, BF16)
                res = []
                for t in range(NT):
                    rt = fin.tile([P, D], FP32, tag="res", bufs=8,
                                  name=f"res{t}")
                    nc.sync.dma_start(rt, q_d[t * P:(t + 1) * P, :])
                    res.append(rt)
                out_acc = [fin.tile([P, D], FP32, tag="oacc", bufs=8,
                                    name=f"oacc{t}")
                           for t in range(NT)]

                def emit_tr(m, pj):
                    """Transpose projT_m blocks into out_acc, fusing the
                    residual add."""
                    for t in range(NT):
                        pt = ps3.tile([P, P], FP32, tag="tr", bufs=4,
                                      name="ps_tr2")
                        nc.tensor.transpose(
                            pt, pj[:, t * P:(t + 1) * P], ident)
                        ms = slice(m * P, (m + 1) * P)
                        nc.vector.tensor_tensor(
                            out_acc[t][:, ms], pt, res[t][:, ms], OP.add)

                prev = None
                for m in range(NC):
                    pj = fin.tile([P, L], FP32, tag="pjt", bufs=3,
                                  name="pjt")
                    for n in range(NQ):
                        ps = ps3.tile([P, QH], FP32, tag="mm", bufs=4,
                                      name="ps_w")
                        for jj in range(NC):
                            nc.tensor.matmul(
                                ps,
                                wo_t[jj][:, m * P:(m + 1) * P],
                                OT[jj][:, n * QH:(n + 1) * QH],
                                start=(jj == 0), stop=(jj == NC - 1),
                            )
                        nc.vector.tensor_copy(pj[:, n * QH:(n + 1) * QH], ps)
                    if prev is not None:
                        emit_tr(m - 1, prev)
                    prev = pj
                emit_tr(NC - 1, prev)

                for t in range(NT):
                    u = out_acc[t]
                    y = fin.tile([P, D], FP32, tag="y", bufs=2, name="y")
                    _emit_layernorm(nc, fin, u, y, gamma_bc, beta_bc, eps_t)
                    nc.sync.dma_start(out_d[t * P:(t + 1) * P, :], y)

    nc.compile()
    return nc


_CACHE = {}


def _get_nc():
    if "nc" not in _CACHE:
        _CACHE["nc"] = build_bass()
    return _CACHE["nc"]


def make_in_maps(q, k, v, Wq, Wk, Wv, Wo, gamma, beta):
    q = np.asarray(q, np.float32)
    k = np.asarray(k, np.float32)
    v = np.asarray(v, np.float32)
    # fold the 1/sqrt(dk) attention scale into Wq (0.125 is exact in fp32)
    wq = (np.asarray(Wq, np.float32) * 0.125).astype(np.float32)
    wk = np.ascontiguousarray(np.asarray(Wk, np.float32))
    wv = np.ascontiguousarray(np.asarray(Wv, np.float32))
    wo = np.asarray(Wo, np.float32).astype(ml_dtypes.bfloat16)
    gb = np.ascontiguousarray(
        np.tile(np.asarray(gamma, np.float32)[None, :], (P, 1)))
    bb = np.ascontiguousarray(
        np.tile(np.asarray(beta, np.float32)[None, :], (P, 1)))
    ident = np.eye(P, dtype=np.float32)
    ones1 = np.ones((1, HD), np.float32)
    epsc = np.full((P, 1), EPS, np.float32)
    vone = np.ones((P, H * E), ml_dtypes.bfloat16)
    zz = np.zeros((HD, L), np.float32)
    B = q.shape[0]
    return [
        {
            "q": np.ascontiguousarray(q[b]),
            "k": np.ascontiguousarray(k[b]),
            "v": np.ascontiguousarray(v[b]),
            "wq": wq, "wk": wk, "wv": wv, "wo": wo,
            "gb": gb, "bb": bb, "ident": ident, "ones1": ones1,
            "epsc": epsc, "vone": vone, "zz": zz,
        }
        for b in range(B)
    ]


def kernel(q, k, v, Wq, Wk, Wv, Wo, gamma, beta, trace=False):
    from concourse.bass_utils import run_bass_kernel_spmd

    nc = _get_nc()
    in_maps = make_in_maps(q, k, v, Wq, Wk, Wv, Wo, gamma, beta)
    res = run_bass_kernel_spmd(nc, in_maps, core_ids=list(range(len(in_maps))),
                               trace=trace)
    out = np.stack([r["out"] for r in res.results], axis=0)
    if trace:
        return out, res
    return out
